# revision 13
# baseline (speedup 1.0000x reference)
"""Trainium2 Bass kernel: GQA attention (H=32, KVH=8, HD=128) with RoPE +
ALiBi + causal mask + output projection, tensor-parallel over heads on 8
NeuronCores.

Contract: kernel(**inputs) takes FULL unsharded inputs (x, wq, wk, wv, wo,
alibi_bias) and returns the FULL (1, 2048, 4096) float32 output.

Per-core plan (core c):
  - owns global q-heads [4c, 4c+4) and kv-head c.
  - host pre-transposes weights and folds the RoPE interleave->split
    permutation into wq/wk rows (so on-device RoPE is two half-partition
    swaps + mul/add), and folds 1/sqrt(HD) into wk.
  - projections run in bf16 (weights + x cast on host), contraction d on
    the partition axis, producing Q^T/K^T [hd, s] directly.
  - scores are computed transposed: S^T[k, q] so that softmax exp output
    P^T [k, q] can feed PV as the stationary operand with no P transposes.
  - ALiBi: when the bias input matches the canonical slope*(k-q) form
    (detected on host), the bias is reconstructed on device from tiny
    per-head rel tiles + per-tile scalar offsets (no 0.5GB streaming);
    otherwise falls back to streaming the host-transposed bias.
  - PV: ctx[q, hd+1] = P^T.T @ [V | ones]; the ones column yields the
    softmax denominator for free. Normalize via reciprocal + per-partition
    scale on the PSUM->SBUF copy, then PE-transpose ctx -> ctx^T [hd, q].
  - out-proj in fp32r; host sums the 8 partial outputs.
"""

import sys

for _p in ("/opt/trn_rl_repo",):
    if _p not in sys.path:
        sys.path.insert(0, _p)

import numpy as np
import ml_dtypes

B, S, D = 1, 2048, 4096
H, KVH = 32, 8
HD = D // H            # 128
NCORES = 8
HPC = H // NCORES      # 4 q heads per core
MQ = HPC * HD          # 512
ROPE_THETA = 10000.0

SC = 512               # projection s-chunk
NSC = S // SC          # 4
QC = 512               # attention q-chunk
NQC = S // QC          # 4
NKT = S // 128         # 16 k-tiles
NDT = D // 128         # 32 d-tiles
NEG = -60000.0         # causal fill (exp -> 0)

_CACHE = {}


def _build_module(structured):
    import concourse.mybir as mybir
    import concourse.tile as tile
    from concourse import bacc
    from concourse.masks import make_identity
    from contextlib import ExitStack

    f32 = mybir.dt.float32
    f32r = mybir.dt.float32r
    bf16 = mybir.dt.bfloat16
    Exp = mybir.ActivationFunctionType.Exp

    nc = bacc.Bacc(trn_type="TRN2")

    xT = nc.dram_tensor("xT", [D, S], bf16, kind="ExternalInput")
    wqT = nc.dram_tensor("wqT", [D, MQ], bf16, kind="ExternalInput")
    wkT = nc.dram_tensor("wkT", [D, HD], bf16, kind="ExternalInput")
    wvT = nc.dram_tensor("wvT", [D, HD], bf16, kind="ExternalInput")
    woT = nc.dram_tensor("woT", [MQ, D], bf16, kind="ExternalInput")
    cosE = nc.dram_tensor("cosE", [HD, S], f32, kind="ExternalInput")
    sinE = nc.dram_tensor("sinE", [HD, S], f32, kind="ExternalInput")
    if structured:
        # srbm[h, r, dk, dq]: r<4 -> masked bias tile for diagonal offset
        # r*128 (full bias incl offset, NEG above diagonal); r=4 -> plain
        # slope*(dk-dq) tile for fully-causal tiles.
        srbm_d = nc.dram_tensor("srbm", [HPC, 5, 128, QC], f32,
                                kind="ExternalInput")
        kbias_d = nc.dram_tensor("kbias", [128, HPC, 16], f32,
                                 kind="ExternalInput")
    else:
        biasT = nc.dram_tensor("biasT", [HPC, S, S], f32, kind="ExternalInput")
    out = nc.dram_tensor("out", [S, D], f32, kind="ExternalOutput")

    with tile.TileContext(nc) as tc, ExitStack() as top:
        persist = top.enter_context(tc.tile_pool(name="persist", bufs=1))

        qt_h = [persist.tile([128, S], f32r, tag=f"qt{h}", name=f"qt{h}")
                for h in range(HPC)]
        kt_t = persist.tile([128, S], f32r, tag="kt")
        vaug = [persist.tile([128, HD + 1], bf16, tag=f"vaug{k}", name=f"vaug{k}")
                for k in range(NKT)]
        ctxT_h = [persist.tile([128, S], bf16, tag=f"ctxT{h}", name=f"ctxT{h}")
                  for h in range(HPC)]
        ident = persist.tile([128, 128], f32, tag="ident")
        identb = persist.tile([128, 128], bf16, tag="identb")
        wq_s = [persist.tile([128, NDT, HD], bf16, tag=f"wq{m}", name=f"wq{m}")
                for m in range(HPC)]
        wk_s = persist.tile([128, NDT, HD], bf16, tag="wk")
        wv_s = persist.tile([128, NDT, HD], bf16, tag="wv")

        make_identity(nc, ident[:])
        make_identity(nc, identb[:])
        wqT_re = wqT[:].rearrange("(a p) m -> p a m", p=128)
        for m in range(HPC):
            nc.sync.dma_start(out=wq_s[m][:],
                              in_=wqT_re[:, :, m * 128:(m + 1) * 128])
        nc.sync.dma_start(out=wk_s[:], in_=wkT[:].rearrange("(a p) m -> p a m", p=128))
        nc.sync.dma_start(out=wv_s[:], in_=wvT[:].rearrange("(a p) m -> p a m", p=128))
        for k in range(NKT):
            nc.vector.memset(vaug[k][:, HD:HD + 1], 1.0)

        if structured:
            sr_t = persist.tile([128, HPC, 5, QC], f32, tag="sr")
            kb_t = persist.tile([128, HPC, 16], f32, tag="kb")

        # ---------------- Phase 1: QKV projections + RoPE ----------------
        with ExitStack() as ph1:
            cspool = ph1.enter_context(tc.tile_pool(name="cspool", bufs=2))
            xpool = ph1.enter_context(tc.tile_pool(name="xpool", bufs=1))
            pp = ph1.enter_context(tc.tile_pool(name="pp", bufs=6, space="PSUM"))
            tpp = ph1.enter_context(tc.tile_pool(name="tpp", bufs=2, space="PSUM"))
            rsc = ph1.enter_context(tc.tile_pool(name="rsc", bufs=2))

            xT_re = xT[:].rearrange("(a p) s -> p a s", p=128)
            for sc in range(NSC):
                s0 = sc * SC
                cos_s = cspool.tile([128, SC], f32, tag="cos")
                sin_s = cspool.tile([128, SC], f32, tag="sin")
                nc.gpsimd.dma_start(out=cos_s[:], in_=cosE[:][:, s0:s0 + SC])
                nc.gpsimd.dma_start(out=sin_s[:], in_=sinE[:][:, s0:s0 + SC])
                xts = []
                for q4 in range(4):
                    xq = xpool.tile([128, NDT // 4, SC], bf16, tag=f"xt{q4}",
                                    name=f"xt{q4}")
                    nc.scalar.dma_start(
                        out=xq[:],
                        in_=xT_re[:, q4 * 8:(q4 + 1) * 8, s0:s0 + SC],
                    )
                    xts.append(xq)
                # m-tiles: 0..HPC-1 = q heads, HPC = k, HPC+1 = v (as vT)
                for m in range(HPC + 2):
                    ps = pp.tile([128, SC], f32, tag="ps")
                    for dt in range(NDT):
                        if m < HPC:
                            lhsT = wq_s[m][:, dt, :]
                        elif m == HPC:
                            lhsT = wk_s[:, dt, :]
                        else:
                            lhsT = wv_s[:, dt, :]
                        nc.tensor.matmul(
                            ps[:], lhsT, xts[dt // 8][:, dt % 8, :],
                            start=(dt == 0), stop=(dt == NDT - 1),
                        )
                    if m <= HPC:
                        # RoPE in split layout: out = t*cosE + swap(t)*sinE
                        dst = qt_h[m] if m < HPC else kt_t
                        pss = rsc.tile([128, SC], f32, tag="pss")
                        nc.scalar.copy(pss[:], ps[:])
                        tc_f = rsc.tile([128, SC], f32, tag="ropecos")
                        nc.vector.tensor_mul(tc_f[:], pss[:], cos_s[:])
                        sw = rsc.tile([128, SC], f32, tag="ropeswap")
                        nc.gpsimd.dma_start(out=sw[0:64, :], in_=pss[64:128, :])
                        nc.gpsimd.dma_start(out=sw[64:128, :], in_=pss[0:64, :])
                        nc.vector.tensor_mul(sw[:], sw[:], sin_s[:])
                        nc.vector.tensor_add(dst[:, s0:s0 + SC], tc_f[:], sw[:])
                    else:
                        # vT [hd, s-chunk] -> transpose into V tiles [k, hd]
                        for j in range(SC // 128):
                            vs = rsc.tile([128, 128], f32, tag="vs")
                            nc.scalar.copy(vs[:], ps[:, j * 128:(j + 1) * 128])
                            tp = tpp.tile([128, 128], f32, tag="tp")
                            nc.tensor.transpose(tp[:], vs[:], ident[:])
                            kti = (s0 // 128) + j
                            nc.vector.tensor_copy(vaug[kti][:, 0:HD], tp[:])

        if structured:
            nc.gpsimd.dma_start(
                out=sr_t[:],
                in_=srbm_d[:].rearrange("h r p q -> p h r q"),
            )
            nc.gpsimd.dma_start(out=kb_t[:], in_=kbias_d[:])

        # ---------------- Phase 2: attention per head ----------------
        with ExitStack() as ph2:
            sp = ph2.enter_context(tc.tile_pool(name="sp", bufs=2, space="PSUM"))
            pvp = ph2.enter_context(tc.tile_pool(name="pvp", bufs=1, space="PSUM"))
            tp2 = ph2.enter_context(tc.tile_pool(name="tp2", bufs=2, space="PSUM"))
            bsc = ph2.enter_context(tc.tile_pool(name="bsc", bufs=3))
            ssc = ph2.enter_context(tc.tile_pool(name="ssc", bufs=3))
            ptp = ph2.enter_context(tc.tile_pool(name="ptp", bufs=4))
            fsc = ph2.enter_context(tc.tile_pool(name="fsc", bufs=3))

            for h in range(HPC):
                for qc in range(NQC):
                    q0 = qc * QC
                    pv = [pvp.tile([128, HD + 1], f32, tag=f"pv{j}", name=f"pv{j}")
                          for j in range(4)]
                    nkt_c = 4 * qc + 4  # k-tiles with any unmasked element
                    for kt in range(nkt_c):
                        st = sp.tile([128, QC], f32, tag="st")
                        nc.tensor.matmul(
                            st[:],
                            kt_t[:, kt * 128:(kt + 1) * 128],
                            qt_h[h][:, q0:q0 + QC],
                            start=True, stop=True,
                        )
                        ss = ssc.tile([128, QC], f32, tag="ss")
                        r = kt - 4 * qc
                        if structured:
                            rr = 4 if r < 0 else r   # 4 = plain SR tile
                            nc.vector.tensor_add(ss[:], st[:], sr_t[:, h, rr, :])
                            ebias = kb_t[:, h, r + 15:r + 16] if r < 0 else 0.0
                        else:
                            bt = bsc.tile([128, QC], f32, tag="bt")
                            nc.sync.dma_start(
                                out=bt[:],
                                in_=biasT[h, kt * 128:(kt + 1) * 128, q0:q0 + QC],
                            )
                            nc.vector.tensor_add(ss[:], st[:], bt[:])
                            ebias = 0.0
                        pt = ptp.tile([128, QC], bf16, tag="pt")
                        if isinstance(ebias, float):
                            nc.scalar.activation(pt[:], ss[:], Exp, bias=ebias)
                        else:
                            nc.scalar.activation(pt[:], ss[:], Exp, bias=ebias)
                        for j in range(4):
                            ktmax = 4 * qc + j
                            if kt <= ktmax:
                                nc.tensor.matmul(
                                    pv[j][:],
                                    pt[:, j * 128:(j + 1) * 128],
                                    vaug[kt][:],
                                    start=(kt == 0), stop=(kt == ktmax),
                                )
                    for j in range(4):
                        rcp = fsc.tile([128, 1], f32, tag="rcp")
                        nc.vector.reciprocal(rcp[:], pv[j][:, HD:HD + 1])
                        cs = fsc.tile([128, 128], bf16, tag="cs")
                        nc.scalar.mul(cs[:], pv[j][:, 0:HD], mul=rcp[:])
                        tp = tp2.tile([128, 128], bf16, tag="tpc")
                        nc.tensor.transpose(tp[:], cs[:], identb[:])
                        col = q0 + j * 128
                        nc.vector.tensor_copy(ctxT_h[h][:, col:col + 128], tp[:])

        # ---------------- Phase 3: output projection ----------------
        with ExitStack() as ph3:
            wop = ph3.enter_context(tc.tile_pool(name="wop", bufs=2))
            op = ph3.enter_context(tc.tile_pool(name="op", bufs=6, space="PSUM"))
            osb = ph3.enter_context(tc.tile_pool(name="osb", bufs=2))

            out_re = out[:].rearrange("(a p) o -> p a o", p=128)
            for oc in range(D // 512):
                wo_t = wop.tile([128, HPC, 512], bf16, tag="wo")
                nc.sync.dma_start(
                    out=wo_t[:],
                    in_=woT[:].rearrange("(a p) o -> p a o", p=128)[
                        :, :, oc * 512:(oc + 1) * 512
                    ],
                )
                for half in range(2):
                    ob = osb.tile([128, 8, 512], f32, tag="ob")
                    for sti in range(8):
                        stt = half * 8 + sti
                        po = op.tile([128, 512], f32, tag="po")
                        for h in range(HPC):
                            nc.tensor.matmul(
                                po[:],
                                ctxT_h[h][:, stt * 128:(stt + 1) * 128],
                                wo_t[:, h, :],
                                start=(h == 0), stop=(h == HPC - 1),
                            )
                        nc.scalar.copy(ob[:, sti, :], po[:])
                    nc.sync.dma_start(
                        out=out_re[:, half * 8:(half + 1) * 8,
                                   oc * 512:(oc + 1) * 512],
                        in_=ob[:],
                    )

    nc.compile()
    return nc


def _detect_structured(alibi_bias):
    """True + slopes if alibi_bias[0,h,q,k] == f32(slope_h * (k-q))."""
    b = alibi_bias[0]
    slopes = b[:, 0, 1].astype(np.float64)  # slope_h * 1
    qs = np.arange(0, S, 97)
    ks = np.arange(0, S, 89)
    rel = (ks[None, :] - qs[:, None]).astype(np.float64)
    want = (slopes[:, None, None] * rel[None]).astype(np.float32)
    got = b[:, qs][:, :, ks]
    return bool(np.array_equal(want, got)), slopes


def _host_inputs(x, wq, wk, wv, wo, alibi_bias, structured, slopes):
    bf16 = ml_dtypes.bfloat16
    x2 = np.ascontiguousarray(x.reshape(S, D))
    xT = np.ascontiguousarray(x2.T).astype(bf16)

    perm = np.concatenate([np.arange(0, HD, 2), np.arange(1, HD, 2)])

    invf = (1.0 / (ROPE_THETA ** (np.arange(0, HD, 2) / HD))).astype(np.float64)
    ang = np.arange(S, dtype=np.float64)[None, :] * invf[:, None]  # (64, S)
    cosE = np.concatenate([np.cos(ang), np.cos(ang)], 0).astype(np.float32)
    sinE = np.concatenate([-np.sin(ang), np.sin(ang)], 0).astype(np.float32)

    if not structured:
        kq = np.arange(S)
        causal_mask = kq[:, None] > kq[None, :]  # [k, q] True above diagonal

    dk = np.arange(128, dtype=np.float64)[:, None]
    dq = np.arange(QC, dtype=np.float64)[None, :]

    in_maps = []
    ebias_all = []
    for c in range(NCORES):
        wq_c = wq[c * MQ:(c + 1) * MQ].reshape(HPC, HD, D)[:, perm, :].reshape(MQ, D)
        wk_c = wk[c * HD:(c + 1) * HD][perm] * (1.0 / np.sqrt(HD))
        wv_c = wv[c * HD:(c + 1) * HD]
        m = {
            "xT": xT,
            "wqT": np.ascontiguousarray(wq_c.T).astype(bf16),
            "wkT": np.ascontiguousarray(wk_c.T).astype(bf16),
            "wvT": np.ascontiguousarray(wv_c.T).astype(bf16),
            "woT": np.ascontiguousarray(wo[:, c * MQ:(c + 1) * MQ].T).astype(bf16),
            "cosE": cosE,
            "sinE": sinE,
        }
        if structured:
            srbm = np.empty((HPC, 5, 128, QC), np.float32)
            for hl in range(HPC):
                sl = slopes[c * HPC + hl]
                srbm[hl, 4] = (sl * (dk - dq)).astype(np.float32)
                for r in range(4):
                    v = sl * (dk - dq + 128.0 * r)
                    v = np.where(dk - dq + 128.0 * r > 0, np.float64(NEG), v)
                    srbm[hl, r] = v.astype(np.float32)
            m["srbm"] = srbm
            kb = np.empty((128, HPC, 16), np.float32)
            for hl in range(HPC):
                sl = slopes[c * HPC + hl]
                for i in range(16):
                    kb[:, hl, i] = np.float32(sl * 128.0 * (i - 15))
            m["kbias"] = kb
        else:
            bias_c = alibi_bias[0, c * HPC:(c + 1) * HPC]
            biasT_c = np.ascontiguousarray(bias_c.transpose(0, 2, 1))
            biasT_c = np.where(causal_mask[None], np.float32(NEG), biasT_c)
            m["biasT"] = np.ascontiguousarray(biasT_c).astype(np.float32)
        in_maps.append(m)
    return in_maps, ebias_all


def kernel(x, wq, wk, wv, wo, alibi_bias):
    import os
    from concourse.bass_utils import run_bass_kernel_spmd

    x = np.asarray(x, dtype=np.float32)
    wq = np.asarray(wq, dtype=np.float32)
    wk = np.asarray(wk, dtype=np.float32)
    wv = np.asarray(wv, dtype=np.float32)
    wo = np.asarray(wo, dtype=np.float32)
    alibi_bias = np.asarray(alibi_bias, dtype=np.float32)

    structured, slopes = _detect_structured(alibi_bias)
    if os.environ.get("KERNEL_FORCE_GENERAL", "0") == "1":
        structured = False

    in_maps, _ = _host_inputs(
        x, wq, wk, wv, wo, alibi_bias, structured, slopes
    )

    key = ("nc", structured)
    if key not in _CACHE:
        _CACHE[key] = _build_module(structured)
    nc = _CACHE[key]

    res = run_bass_kernel_spmd(nc, in_maps, core_ids=list(range(NCORES)))
    _CACHE["last_exec_ns"] = res.exec_time_ns
    _CACHE["last_results"] = res

    acc = res.results[0]["out"].astype(np.float64)
    for c in range(1, NCORES):
        acc += res.results[c]["out"]
    return acc.astype(np.float32).reshape(B, S, D)
